# revision 1
# baseline (speedup 1.0000x reference)
"""Trainium2 Bass kernel for Deimv2 LQE (softmax -> top4 -> stat -> MLP -> +scores).

Contract: kernel(**inputs) takes FULL unsharded numpy inputs
(scores [64,4096,1], pred_corners [64,4096,132], w1 [20,64], b1 [64],
w2 [64,1], b2 [1]) and returns the full [64,4096,1] float32 output.
Internally shards batch across 8 NeuronCores (pure data parallel).

Math notes (v2 — engine-balanced rewrite of the v1 baseline):
  - softmax over 33 bins/corner without max-subtraction (inputs ~N(0,1)).
  - top4(softmax(x)) == top4(exp(x)) / sum(exp(x)); sorted top-8 via DVE Max,
    lanes 0..3. The 64 Max ops/tile (~6.1us) are the DVE floor, so everything
    else moves off DVE:
      * denominator sum(exp) runs on the Pool/GPSIMD engine as a 6-op
        pairwise-add tree (33 = 16+16+1), not DVE reduce_sum;
      * stat normalize-multiply runs on Pool;
      * DVE keeps only the 64 Max + one reciprocal per tile.
  - mean stat slot folded into W1 (w1'[c*5+i] = w1[c*5+i] + w1[c*5+4]/4).
  - MLP matmuls run in bf16 (stat cast at the PSUM->SBUF copy, relu output
    cast to bf16): 1 cycle/row on PE vs 4 for fp32. Scores enter the final
    PSUM accumulation as an identity matmul in full f32; b2 is added by the
    ACT copy's bias port. stat tiles are persistent with one-time zero fill
    (mean slots stay zero; only top-4 slots are rewritten each tile).
  - Software pipeline is two-deep: stage1(i) = {exp-dependent Pool tree-sum,
    DVE Max, DVE reciprocal}; stage2(i-1) = {Pool mul, PE transpose+MLP, ACT
    copies, DMA out} so the Pool mul never waits on the same-tile reciprocal
    and each engine sees a clean in-order queue.
"""

import sys

for _p in ("/opt/trn_rl_repo", "/opt/trn_rl_repo/concourse"):
    if _p not in sys.path:
        sys.path.insert(0, _p)

import numpy as np

import concourse.bass as bass
import concourse.mybir as mybir
import concourse.tile as tile
from concourse import masks
from concourse.bass_utils import run_bass_kernel_spmd

# Problem shape (hardcoded per contract)
B, L = 64, 4096
NBINS1 = 33          # bins per corner
C = 4                # corners
D = C * NBINS1       # 132
TOPK = 4
HID = 64
IN_DIM = C * (TOPK + 1)  # 20
N_CORES = 8
R = B * L // N_CORES     # rows per core = 32768
G = R // 128             # row-groups per core = 256
T = 16                   # row-groups per tile
TC = T * C               # corner-instances per partition per tile = 64
NTILES = G // T          # 16
H = T // 8               # half-tile count (1024-row MLP slabs per tile)
NBT = T // 4             # transpose batches (4 groups each) per tile
GP = 32                  # padded stat dims per group (20 used + 12 zero)

F32 = mybir.dt.float32
BF16 = mybir.dt.bfloat16


def _split_waits(nc, max_waits=1):
    """This walrus build rejects instructions with >1 sync wait; move excess
    waits onto preceding same-engine NoOps (engine order preserves them)."""
    n_new = 0
    for f in nc.m.functions:
        for bb in f.blocks:
            insts = bb.instructions
            i = 0
            while i < len(insts):
                inst = insts[i]
                si = inst.sync_info
                if si is not None and len(si.on_wait) > max_waits:
                    waits = list(si.on_wait)
                    keep, rest = waits[:max_waits], waits[max_waits:]
                    nops = []
                    while rest:
                        chunk, rest = rest[:max_waits], rest[max_waits:]
                        nop = mybir.InstNoOp(
                            name=f"I-waitsplit-{n_new}", ins=[], outs=[]
                        )
                        n_new += 1
                        nop.engine = inst.engine
                        nop.sync_info = mybir.SyncInfo(on_wait=chunk, on_update=[])
                        nops.append(nop)
                    inst.sync_info = mybir.SyncInfo(
                        on_wait=keep, on_update=list(si.on_update)
                    )
                    for j, nop in enumerate(nops):
                        insts.insert(i + j, nop)
                    i += len(nops)
                i += 1
    return n_new


def build_kernel(niter=1, ablate=frozenset()):
    nc = bass.Bass(trn_type="TRN2")
    pc = nc.dram_tensor("pc", [R, D], F32, kind="ExternalInput")
    sc = nc.dram_tensor("sc", [R, 1], F32, kind="ExternalInput")
    w1 = nc.dram_tensor("w1", [IN_DIM, HID], F32, kind="ExternalInput")
    b1 = nc.dram_tensor("b1", [HID, 1], F32, kind="ExternalInput")
    w2 = nc.dram_tensor("w2", [HID, 1], F32, kind="ExternalInput")
    b2 = nc.dram_tensor("b2", [1, 1], F32, kind="ExternalInput")
    out = nc.dram_tensor("out", [R, 1], F32, kind="ExternalOutput")

    with tile.TileContext(nc) as tc:
        with (
            tc.tile_pool(name="singles", bufs=1) as singles,
            tc.tile_pool(name="xin", bufs=4) as xin_pool,
            tc.tile_pool(name="ex", bufs=3) as ex_pool,
            tc.tile_pool(name="sums", bufs=2) as sum_pool,
            tc.tile_pool(name="small", bufs=3) as small_pool,
            tc.tile_pool(name="stt", bufs=3) as stt_pool,
            tc.tile_pool(name="hsb", bufs=3) as hs_pool,
            tc.tile_pool(name="scin", bufs=3) as sc_pool,
            tc.tile_pool(name="pst", bufs=2, space="PSUM") as pst_psum,
            tc.tile_pool(name="ph", bufs=2, space="PSUM") as h_psum,
            tc.tile_pool(name="pq", bufs=3, space="PSUM") as q_psum,
        ):
            # ---- main-loop tile pools and helpers (declared up front so the
            # first input DMAs can be issued BEFORE the constant-setup DMAs,
            # which otherwise hold the HWDGE queue for ~10us of startup) ----
            x_tiles = {}
            e_tiles = {}
            t8_tiles = {}
            r_tiles = {}

            def dma_tile(it):
                # two half-tile DMAs + (in exp_tile) two half-tile exps, so
                # the first Max batch of a tile starts one half-DMA earlier
                x = xin_pool.tile([128, T * D], F32, tag="x")
                x_tiles[it] = x
                if "dma" not in ablate:
                    GH = T // 2
                    for h in range(2):
                        rows = it * T * 128 + h * GH * 128
                        nc.sync.dma_start(
                            out=x[:].rearrange("p (g d) -> p g d", d=D)[
                                :, h * GH : (h + 1) * GH, :
                            ],
                            in_=pc[rows : rows + GH * 128, :].rearrange(
                                "(g p) d -> p g d", p=128
                            ),
                        )
                else:
                    nc.gpsimd.memset(x[:, 0:1], 0.0)

            if niter == 1:
                dma_tile(0)
                if NTILES > 1:
                    dma_tile(1)

            # ---- one-time constants ----
            ident = singles.tile([128, 128], F32)
            masks.make_identity(nc, ident[:])

            # Fold the (linear) mean stat slot into W1:
            # w1'[c*5+i] = w1[c*5+i] + w1[c*5+4]/4; mean slots stay zero.
            w1eff = singles.tile([IN_DIM, HID], F32)
            w1mean = singles.tile([IN_DIM, HID], F32)
            nc.sync.dma_start(out=w1eff[:, :], in_=w1[:, :])
            for c in range(C):
                mrow = w1[c * (TOPK + 1) + TOPK, :]  # [64]
                bc = bass.AP(
                    tensor=mrow.tensor,
                    offset=mrow.offset,
                    ap=[[0, TOPK + 1], [1, HID]],
                )
                nc.sync.dma_start(
                    out=w1mean[c * (TOPK + 1) : (c + 1) * (TOPK + 1), :], in_=bc
                )
            nc.vector.scalar_tensor_tensor(
                out=w1eff[:, :],
                in0=w1mean[:, :],
                scalar=1.0 / TOPK,
                in1=w1eff[:, :],
                op0=mybir.AluOpType.mult,
                op1=mybir.AluOpType.add,
            )
            w1eff_bf = singles.tile([IN_DIM, HID], BF16)
            nc.scalar.copy(out=w1eff_bf[:, :], in_=w1eff[:, :])

            w1dA = singles.tile([128, 128], BF16)
            w1dB = singles.tile([128, 128], BF16)
            nc.gpsimd.memset(w1dA[:], 0.0)
            nc.gpsimd.memset(w1dB[:], 0.0)
            # block-sparse W1: group gg in the 4-group transpose batch sits at
            # stat rows gg*32..gg*32+20; groups (0,1)->A cols (0:64, 64:128),
            # groups (2,3)->B cols (0:64, 64:128)
            nc.sync.dma_start(out=w1dA[0:IN_DIM, 0:HID], in_=w1eff_bf[:, :])
            nc.sync.dma_start(out=w1dA[32 : 32 + IN_DIM, HID:128], in_=w1eff_bf[:, :])
            nc.sync.dma_start(out=w1dB[64 : 64 + IN_DIM, 0:HID], in_=w1eff_bf[:, :])
            nc.sync.dma_start(out=w1dB[96 : 96 + IN_DIM, HID:128], in_=w1eff_bf[:, :])

            b1_2 = singles.tile([128, 1], F32)
            nc.sync.dma_start(out=b1_2[0:HID, :], in_=b1[:, :])
            nc.sync.dma_start(out=b1_2[HID:128, :], in_=b1[:, :])

            w2sb = singles.tile([HID, 1], F32)
            nc.sync.dma_start(out=w2sb[:, :], in_=w2[:, :])
            w2d = singles.tile([128, 2], BF16)
            nc.gpsimd.memset(w2d[:], 0.0)
            nc.scalar.copy(out=w2d[0:HID, 0:1], in_=w2sb[:, :])
            nc.scalar.copy(out=w2d[HID:128, 1:2], in_=w2sb[:, :])

            b2_2 = singles.tile([2, 1], F32)
            nc.sync.dma_start(out=b2_2[0:1, :], in_=b2[:, :])
            nc.sync.dma_start(out=b2_2[1:2, :], in_=b2[:, :])

            eye2 = singles.tile([2, 2], F32)
            masks.make_identity(nc, eye2[:])

            # persistent stat tiles: zero-filled once; per tile only the
            # top-4 slots are rewritten (mean + pad slots stay zero forever)
            stat_bufs = [
                singles.tile([128, T, GP], F32, name=f"statbuf{k}") for k in range(2)
            ]
            for sb in stat_bufs:
                nc.gpsimd.memset(sb[:], 0.0)

            # ---- two-deep software pipeline ----
            def exp_tile(it):
                x = x_tiles.pop(it)
                if "exp" not in ablate:
                    e = ex_pool.tile([128, T * D], F32, tag="e")
                    # high_priority: the scheduler must slot exp ahead of the
                    # previous tile's PSUM-copy chain on the in-order ACT
                    # queue, or every engine downstream of exp stalls.
                    HD = (T // 2) * D
                    with tc.high_priority():
                        for h in range(2):
                            nc.scalar.activation(
                                out=e[:, h * HD : (h + 1) * HD],
                                in_=x[:, h * HD : (h + 1) * HD],
                                func=mybir.ActivationFunctionType.Exp,
                            )
                    e_tiles[it] = e
                else:
                    e_tiles[it] = x

            def stage1(it):
                """exp(it)-dependent work: Pool tree-sum, DVE Max + recip.

                Everything runs at half-tile granularity (32 corner-instances)
                so downstream consumers unblock at half-tile boundaries: the
                reciprocal for half h lands right after h's 32 Max ops, and
                the stage2 chain for h overlaps the other half's Max batch.
                """
                e = e_tiles.pop(it)
                e3 = e[:].rearrange("p (tc nb) -> p tc nb", nb=NBINS1)  # [128,64,33]

                t8 = small_pool.tile([128, TC, 8], F32, tag="t8")
                t8_tiles[it] = t8
                r = small_pool.tile([128, TC], F32, tag="r")
                r_tiles[it] = r

                HC = TC // 2  # corner-instances per half
                for h in range(2):
                    e3h = e3[:, h * HC : (h + 1) * HC, :]

                    # Pool: sum over 33 bins = 16+16 pairwise tree + odd bin.
                    # (plain tensor_add: walrus rejects ScalarTensorTensor on
                    # the Pool engine, so the fused form is not available)
                    if "sums" not in ablate:
                        s16 = sum_pool.tile([128, HC, 16], F32, tag="s16")
                        nc.gpsimd.tensor_add(s16[:], e3h[:, :, 0:16], e3h[:, :, 16:32])
                        s8 = sum_pool.tile([128, HC, 8], F32, tag="s8")
                        nc.gpsimd.tensor_add(s8[:], s16[:, :, 0:8], s16[:, :, 8:16])
                        s4 = sum_pool.tile([128, HC, 4], F32, tag="s4")
                        nc.gpsimd.tensor_add(s4[:], s8[:, :, 0:4], s8[:, :, 4:8])
                        s2 = sum_pool.tile([128, HC, 2], F32, tag="s2")
                        nc.gpsimd.tensor_add(s2[:], s4[:, :, 0:2], s4[:, :, 2:4])
                        z = small_pool.tile([128, HC], F32, tag="z")
                        nc.gpsimd.tensor_add(z[:], s2[:, :, 0], s2[:, :, 1])
                        z2 = small_pool.tile([128, HC], F32, tag="z2")
                        nc.gpsimd.tensor_add(z2[:], z[:], e3h[:, :, 32])
                    else:
                        z2 = small_pool.tile([128, HC], F32, tag="z2")
                        nc.gpsimd.memset(z2[:], 1.0)

                    # DVE: sorted top-8 per corner (use lanes 0..3), then the
                    # half's reciprocal right after its Max batch.
                    if "max" not in ablate:
                        for tcb in range(h * HC, (h + 1) * HC):
                            nc.vector.max(out=t8[:, tcb, :], in_=e3[:, tcb, :])
                    elif h == 0:
                        nc.vector.memset(t8[:], 0.125)
                    nc.vector.reciprocal(
                        out=r[:, h * HC : (h + 1) * HC], in_=z2[:]
                    )

            def stage2(it):
                """recip(it)-dependent work at half-tile granularity: Pool
                mul, PE transpose+MLP, ACT copies, one DMA out per tile."""
                base = it * T * 128
                t8 = t8_tiles.pop(it)
                r = r_tiles.pop(it)
                statj = stat_bufs[it % 2]
                GH = T // 2  # groups per half

                statc = statj[:, :, 0:IN_DIM].rearrange(
                    "p g (c s) -> p g c s", s=TOPK + 1
                )

                # scores for the whole tile in one DMA: [2, 8, 128]
                scT = sc_pool.tile([2, H * C, 128], F32, tag="scT")
                nc.sync.dma_start(
                    out=scT[:],
                    in_=sc[base : base + T * 128, :].rearrange(
                        "(h q two p) one -> two (h q) (p one)", two=2, p=128, h=H
                    ),
                )

                qsb = sc_pool.tile([2, H, 512], F32, tag="q_sb")
                for hb in range(H):  # half-tiles of 8 groups / 1024 rows
                    # Pool: normalize the half's top-4 into the stat slots.
                    # (tensor_mul, not scalar_tensor_tensor: walrus rejects
                    # ScalarTensorTensor with 4D output APs)
                    hcs = slice(hb * GH * C, (hb + 1) * GH * C)
                    r3 = r[:, hcs].rearrange("p (g c) -> p g c", c=C)
                    nc.gpsimd.tensor_mul(
                        statc[:, hb * GH : (hb + 1) * GH, :, 0:TOPK],
                        t8[:, hcs, 0:TOPK].rearrange("p (g c) k -> p g c k", c=C),
                        r3.to_broadcast([128, GH, C, TOPK]),
                    )

                    # 2 transposes into one PSUM bank, one psum->sbuf copy
                    # (casting to bf16 for the PE matmuls)
                    sT_ps = pst_psum.tile([128, 256], F32, tag="sT")
                    for bb in range(2):
                        bt = hb * 2 + bb
                        nc.tensor.transpose(
                            out=sT_ps[:, bb * 128 : (bb + 1) * 128],
                            in_=statj[:, bt * 4 : (bt + 1) * 4, :].rearrange(
                                "p g k -> p (g k)"
                            ),
                            identity=ident[:],
                        )
                    sT = stt_pool.tile([128, 256], BF16, tag="sT_sb")
                    nc.scalar.copy(out=sT[:], in_=sT_ps[:])

                    hT = h_psum.tile([128, 512], F32, tag="hT")
                    for bb in range(2):
                        nc.tensor.matmul(
                            out=hT[:, (bb * 2) * 128 : (bb * 2 + 1) * 128],
                            lhsT=w1dA[:],
                            rhs=sT[:, bb * 128 : (bb + 1) * 128],
                        )
                        nc.tensor.matmul(
                            out=hT[:, (bb * 2 + 1) * 128 : (bb * 2 + 2) * 128],
                            lhsT=w1dB[:],
                            rhs=sT[:, bb * 128 : (bb + 1) * 128],
                        )
                    hs = hs_pool.tile([128, 512], BF16, tag="hs")
                    nc.scalar.activation(
                        out=hs[:],
                        in_=hT[:],
                        func=mybir.ActivationFunctionType.Relu,
                        bias=b1_2[:],
                    )

                    q = q_psum.tile([2, 512], F32, tag="q")
                    nc.tensor.matmul(
                        out=q[:], lhsT=w2d[:], rhs=hs[:], start=True, stop=False
                    )
                    nc.tensor.matmul(
                        out=q[:],
                        lhsT=eye2[:],
                        rhs=scT[:, hb * C : (hb + 1) * C, :].rearrange(
                            "t q p -> t (q p)"
                        ),
                        start=False,
                        stop=True,
                    )
                    # PSUM -> SBUF with the +b2 bias on the ACT port
                    nc.scalar.activation(
                        out=qsb[:, hb, :],
                        in_=q[:],
                        func=mybir.ActivationFunctionType.Identity,
                        bias=b2_2[:],
                    )
                nc.sync.dma_start(
                    out=out[base : base + T * 128, :].rearrange(
                        "(h q two p) one -> two (h q) (p one)", two=2, p=128, h=H
                    ),
                    in_=qsb[:].rearrange("t h (q p) -> t (h q) p", p=128),
                )

            def staged_tiles(skip_first_dma=False):
                if not skip_first_dma:
                    dma_tile(0)
                    if NTILES > 1:
                        dma_tile(1)
                exp_tile(0)
                for i in range(NTILES):
                    if i + 2 < NTILES:
                        dma_tile(i + 2)
                    if i + 1 < NTILES:
                        exp_tile(i + 1)
                    # stage2 of the previous tile is emitted BEFORE stage1 of
                    # this one so the Pool mul isn't queued behind this tile's
                    # 5.5us fold tree on the in-order Pool engine.
                    if i >= 1:
                        stage2(i - 1)
                    stage1(i)
                stage2(NTILES - 1)

            if niter == 1:
                staged_tiles(skip_first_dma=True)
            else:
                # hardware loop: body emitted once, executed niter times
                # (bench-only path for on-device timing via slope)
                with tc.For_i(0, niter, 1):
                    staged_tiles()

    _split_waits(nc)
    return nc


_CACHE = {}


def kernel(scores, pred_corners, w1, b1, w2, b2):
    if "nc" not in _CACHE:
        _CACHE["nc"] = build_kernel()
    nc = _CACHE["nc"]

    pc_full = np.ascontiguousarray(
        pred_corners.reshape(B * L, D).astype(np.float32, copy=False)
    )
    sc_full = np.ascontiguousarray(
        scores.reshape(B * L, 1).astype(np.float32, copy=False)
    )
    w1_a = np.ascontiguousarray(w1.astype(np.float32, copy=False))
    b1_a = np.ascontiguousarray(b1.astype(np.float32, copy=False)).reshape(HID, 1)
    w2_a = np.ascontiguousarray(w2.astype(np.float32, copy=False)).reshape(HID, 1)
    b2_a = np.ascontiguousarray(b2.astype(np.float32, copy=False)).reshape(1, 1)

    in_maps = []
    for c in range(N_CORES):
        in_maps.append(
            {
                "pc": pc_full[c * R : (c + 1) * R],
                "sc": sc_full[c * R : (c + 1) * R],
                "w1": w1_a,
                "b1": b1_a,
                "w2": w2_a,
                "b2": b2_a,
            }
        )
    res = run_bass_kernel_spmd(nc, in_maps, core_ids=list(range(N_CORES)))
    outs = [res.results[c]["out"] for c in range(N_CORES)]
    full = np.concatenate(outs, axis=0).reshape(B, L, 1)
    return full



# revision 2
# speedup vs baseline: 1.1225x; 1.1225x over previous
"""Trainium2 Bass kernel for Deimv2 LQE (softmax -> top4 -> stat -> MLP -> +scores).

v3 — DVE-bound rewrite of v2:
  - top-8 runs on RAW LOGITS (exp is monotone), so DVE Max no longer waits on
    exp; stat slots are computed as exp(top4_logit - ln Z) which kills the
    DVE reciprocal and the Pool normalize-mul.
  - Z = sum(exp) via Pool pairwise tree (full-tile ops), lnZ on ACT (the
    natural_log_exp_and_others table holds Exp+Ln+Relu+Identity — no table
    switches).
  - ONE input DMA per tile (was 2 half-DMAs): each HWDGE dma_start costs
    ~1.2us of SP sequencer occupancy regardless of size; SP was 94% busy in
    the v2 cost-model profile (the real bottleneck).
  - mean stat slot folded into W1; MLP in bf16; scores enter via identity
    matmul; b2 via ACT bias port (unchanged from v2).

Engine budget per tile (16 tiles of 2048 rows/core): DVE 6.07us (64 Max8,
binding), ACT ~5.1us (exp, ln, exp4, psum copies), PE ~3us, Pool ~2.2us,
SP ~3.6us (3 DMAs).
"""

import sys

for _p in ("/opt/trn_rl_repo", "/opt/trn_rl_repo/concourse"):
    if _p not in sys.path:
        sys.path.insert(0, _p)

import numpy as np

import concourse.bass as bass
import concourse.mybir as mybir
import concourse.tile as tile
from concourse import masks
from concourse.bass_utils import run_bass_kernel_spmd

# Problem shape (hardcoded per contract)
B, L = 64, 4096
NBINS1 = 33          # bins per corner
C = 4                # corners
D = C * NBINS1       # 132
TOPK = 4
HID = 64
IN_DIM = C * (TOPK + 1)  # 20
N_CORES = 8
R = B * L // N_CORES     # rows per core = 32768
G = R // 128             # row-groups per core = 256
T = 16                   # row-groups per tile
TC = T * C               # corner-instances per partition per tile = 64
NTILES = G // T          # 16
H = T // 8               # half-tile count (1024-row MLP slabs per tile)
GP = 32                  # padded stat dims per group (20 used + 12 zero)

F32 = mybir.dt.float32
BF16 = mybir.dt.bfloat16


def _split_waits(nc, max_waits=1):
    """This walrus build rejects instructions with >1 sync wait; move excess
    waits onto preceding same-engine NoOps (engine order preserves them)."""
    n_new = 0
    for f in nc.m.functions:
        for bb in f.blocks:
            insts = bb.instructions
            i = 0
            while i < len(insts):
                inst = insts[i]
                si = inst.sync_info
                if si is not None and len(si.on_wait) > max_waits:
                    waits = list(si.on_wait)
                    keep, rest = waits[:max_waits], waits[max_waits:]
                    nops = []
                    while rest:
                        chunk, rest = rest[:max_waits], rest[max_waits:]
                        nop = mybir.InstNoOp(
                            name=f"I-waitsplit-{n_new}", ins=[], outs=[]
                        )
                        n_new += 1
                        nop.engine = inst.engine
                        nop.sync_info = mybir.SyncInfo(on_wait=chunk, on_update=[])
                        nops.append(nop)
                    inst.sync_info = mybir.SyncInfo(
                        on_wait=keep, on_update=list(si.on_update)
                    )
                    for j, nop in enumerate(nops):
                        insts.insert(i + j, nop)
                    i += len(nops)
                i += 1
    return n_new


def build_kernel(niter=1):
    # All MLP constants are pre-transformed on the host (fold of the mean
    # stat slot into W1, block-sparse bf16 W1 layouts, duplicated biases,
    # identity matrices) so the device does zero constant-setup compute.
    nc = bass.Bass(trn_type="TRN2")
    pc = nc.dram_tensor("pc", [R, D], F32, kind="ExternalInput")
    sc = nc.dram_tensor("sc", [R, 1], F32, kind="ExternalInput")
    w1dA_in = nc.dram_tensor("w1dA", [128, 128], BF16, kind="ExternalInput")
    w1dB_in = nc.dram_tensor("w1dB", [128, 128], BF16, kind="ExternalInput")
    b1_in = nc.dram_tensor("b1d", [128, 1], F32, kind="ExternalInput")
    w2_in = nc.dram_tensor("w2d", [128, 2], BF16, kind="ExternalInput")
    b2_in = nc.dram_tensor("b2d", [2, 1], F32, kind="ExternalInput")
    ident_in = nc.dram_tensor("identd", [128, 128], F32, kind="ExternalInput")
    eye2_in = nc.dram_tensor("eye2d", [2, 2], F32, kind="ExternalInput")
    out = nc.dram_tensor("out", [R, 1], F32, kind="ExternalOutput")

    with tile.TileContext(nc) as tc:
        with (
            tc.tile_pool(name="singles", bufs=1) as singles,
            tc.tile_pool(name="xin", bufs=4) as xin_pool,
            tc.tile_pool(name="ex", bufs=2) as ex_pool,
            tc.tile_pool(name="sums", bufs=2) as sum_pool,
            tc.tile_pool(name="small", bufs=3) as small_pool,
            tc.tile_pool(name="stt", bufs=3) as stt_pool,
            tc.tile_pool(name="hsb", bufs=3) as hs_pool,
            tc.tile_pool(name="scin", bufs=5) as sc_pool,
            tc.tile_pool(name="qsb", bufs=3) as qsb_pool,
            tc.tile_pool(name="pst", bufs=1, space="PSUM") as pst_psum,
            tc.tile_pool(name="ph", bufs=2, space="PSUM") as h_psum,
            tc.tile_pool(name="pq", bufs=2, space="PSUM") as q_psum,
        ):
            # ---- main-loop pools/helpers declared up front so the first
            # input DMAs beat the constant-setup DMAs into the HWDGE queue ----
            x_tiles = {}
            e_tiles = {}
            t8_tiles = {}
            z2_tiles = {}
            sc_tiles = {}
            q_tiles = {}

            def dma_tile(it, pieces=1):
                # pieces>1: split the transfer by row-group range so the first
                # Max8 ops unblock before the whole tile lands (startup only).
                if it in x_tiles or it >= NTILES:
                    return
                x = xin_pool.tile([128, T * D], F32, tag="x")
                x_tiles[it] = x
                gp = T // pieces
                for pc_i in range(pieces):
                    rows = it * T * 128 + pc_i * gp * 128
                    nc.sync.dma_start(
                        out=x[:].rearrange("p (g d) -> p g d", d=D)[
                            :, pc_i * gp : (pc_i + 1) * gp, :
                        ],
                        in_=pc[rows : rows + gp * 128, :].rearrange(
                            "(g p) d -> p g d", p=128
                        ),
                    )

            def scT_dma(it):
                if it in sc_tiles:
                    return
                base = it * T * 128
                scT = sc_pool.tile([2, H * C, 128], F32, tag="scT")
                nc.sync.dma_start(
                    out=scT[:],
                    in_=sc[base : base + T * 128, :].rearrange(
                        "(h q two p) one -> two (h q) (p one)", two=2, p=128, h=H
                    ),
                )
                sc_tiles[it] = scT

            def prefetch():
                # tiles 0-3 + scores 0-2 are issued before everything else so
                # the constant-setup DMAs never sit in front of them on SP
                dma_tile(0, pieces=4)
                dma_tile(1)
                scT_dma(0)
                dma_tile(2)
                scT_dma(1)
                dma_tile(3)
                scT_dma(2)

            if niter == 1:
                prefetch()

            # persistent stat tiles: zero-filled once; per tile only the
            # top-4 slots are rewritten (mean + pad slots stay zero forever)
            stat_bufs = [
                singles.tile([128, T, GP], F32, name=f"statbuf{k}") for k in range(2)
            ]
            for sb in stat_bufs:
                nc.gpsimd.memset(sb[:], 0.0)

            ident = singles.tile([128, 128], F32)
            eye2 = singles.tile([2, 2], F32)
            w1dA = singles.tile([128, 128], BF16)
            w1dB = singles.tile([128, 128], BF16)
            b1_2 = singles.tile([128, 1], F32)
            w2d = singles.tile([128, 2], BF16)
            b2_2 = singles.tile([2, 1], F32)

            def constants_b():
                """Host-precomputed MLP constants: 7 dependency-free DMAs,
                emitted after stage1(0) so they queue behind the prefetches."""
                nc.sync.dma_start(out=w1dA[:, :], in_=w1dA_in[:, :])
                nc.sync.dma_start(out=w1dB[:, :], in_=w1dB_in[:, :])
                nc.sync.dma_start(out=b1_2[:, :], in_=b1_in[:, :])
                nc.sync.dma_start(out=w2d[:, :], in_=w2_in[:, :])
                nc.sync.dma_start(out=b2_2[:, :], in_=b2_in[:, :])
                nc.sync.dma_start(out=ident[:, :], in_=ident_in[:, :])
                nc.sync.dma_start(out=eye2[:, :], in_=eye2_in[:, :])

            # ---- two-deep software pipeline ----
            def exp_tile(it):
                """ACT exp(x) — emitted ahead of stage2(it-1) so the in-order
                ACT queue finishes exp(it) early; shortens the pipeline tail."""
                x = x_tiles[it]
                e = ex_pool.tile([128, T * D], F32, tag="e")
                nc.scalar.activation(
                    out=e[:],
                    in_=x[:],
                    func=mybir.ActivationFunctionType.Exp,
                )
                e_tiles[it] = e

            def stage1(it):
                """DMA(it)-dependent work: DVE Max8 on raw logits; Pool
                pairwise tree-sum for Z; scores DMA prefetch."""
                x = x_tiles.pop(it)
                x3 = x[:].rearrange("p (tc nb) -> p tc nb", nb=NBINS1)  # [128,64,33]

                t8 = small_pool.tile([128, TC, 8], F32, tag="t8")
                t8_tiles[it] = t8
                for tcb in range(TC):
                    nc.vector.max(out=t8[:, tcb, :], in_=x3[:, tcb, :])

                e = e_tiles.pop(it)
                e3 = e[:].rearrange("p (tc nb) -> p tc nb", nb=NBINS1)
                # Pool: Z = sum over 33 bins = 16+16 pairwise tree + odd bin.
                s16 = sum_pool.tile([128, TC, 16], F32, tag="s16")
                nc.gpsimd.tensor_add(s16[:], e3[:, :, 0:16], e3[:, :, 16:32])
                s8 = sum_pool.tile([128, TC, 8], F32, tag="s8")
                nc.gpsimd.tensor_add(s8[:], s16[:, :, 0:8], s16[:, :, 8:16])
                s4 = sum_pool.tile([128, TC, 4], F32, tag="s4")
                nc.gpsimd.tensor_add(s4[:], s8[:, :, 0:4], s8[:, :, 4:8])
                s2 = sum_pool.tile([128, TC, 2], F32, tag="s2")
                nc.gpsimd.tensor_add(s2[:], s4[:, :, 0:2], s4[:, :, 2:4])
                z = small_pool.tile([128, TC], F32, tag="z")
                nc.gpsimd.tensor_add(z[:], s2[:, :, 0], s2[:, :, 1])
                z2 = small_pool.tile([128, TC], F32, tag="z2")
                nc.gpsimd.tensor_add(z2[:], z[:], e3[:, :, 32])
                z2_tiles[it] = z2

            def stage2(it, split=False):
                """Z(it)-dependent work: lnZ, stat=exp(top4-lnZ), MLP.

                split=True (last tile): half-tile granularity so the hb0
                chain starts as soon as the first 32 Max ops finish, instead
                of waiting for the whole 64-Max batch — shortens the tail."""
                t8 = t8_tiles.pop(it)
                z2 = z2_tiles.pop(it)
                scT = sc_tiles.pop(it)
                statj = stat_bufs[it % 2]

                lnz = small_pool.tile([128, TC], F32, tag="lnz")
                nc.scalar.activation(
                    out=lnz[:],
                    in_=z2[:],
                    func=mybir.ActivationFunctionType.Ln,
                )

                statc = statj[:, :, 0:IN_DIM].rearrange(
                    "p g (c s) -> p g c s", s=TOPK + 1
                )
                tmp4 = small_pool.tile([128, TC, TOPK], F32, tag="tmp4")
                sT_ps = pst_psum.tile([128, 512], F32, tag="sT")
                sT = stt_pool.tile([128, 512], BF16, tag="sT_sb")
                hT = h_psum.tile([128, 1024], F32, tag="hT")
                hs = hs_pool.tile([128, 1024], BF16, tag="hs")
                qsb = qsb_pool.tile([2, H, 512], F32, tag="q_sb")

                HC = TC // 2
                GH = T // 2

                for part in range(H if split else 1):
                    if split:
                        cs = slice(part * HC, (part + 1) * HC)  # corner-insts
                        gs = slice(part * GH, (part + 1) * GH)  # groups
                        bts = range(part * 2, part * 2 + 2)
                        hbs = [part]
                        fs = slice(part * 512, (part + 1) * 512)
                    else:
                        cs = slice(0, TC)
                        gs = slice(0, T)
                        bts = range(4)
                        hbs = list(range(H))
                        fs = slice(0, 1024)

                    nc.gpsimd.tensor_tensor(
                        out=tmp4[:, cs, :],
                        in0=t8[:, cs, 0:TOPK],
                        in1=lnz[:, cs].to_broadcast(
                            [128, cs.stop - cs.start, TOPK]
                        ),
                        op=mybir.AluOpType.subtract,
                    )
                    nc.scalar.activation(
                        out=statc[:, gs, :, 0:TOPK],
                        in_=tmp4[:, cs, :].rearrange("p (g c) k -> p g c k", c=C),
                        func=mybir.ActivationFunctionType.Exp,
                    )

                    # transposes into one PSUM bank, one psum->sbuf copy
                    # (casting to bf16 for the PE matmuls)
                    for bt in bts:
                        nc.tensor.transpose(
                            out=sT_ps[:, bt * 128 : (bt + 1) * 128],
                            in_=statj[:, bt * 4 : (bt + 1) * 4, :].rearrange(
                                "p g k -> p (g k)"
                            ),
                            identity=ident[:],
                        )
                    nc.scalar.copy(
                        out=sT[:, fs.start // 2 : fs.stop // 2],
                        in_=sT_ps[:, fs.start // 2 : fs.stop // 2],
                    )

                    # W1 matmuls into the 2-bank hT, one relu
                    for bt in bts:
                        nc.tensor.matmul(
                            out=hT[:, (bt * 2) * 128 : (bt * 2 + 1) * 128],
                            lhsT=w1dA[:],
                            rhs=sT[:, bt * 128 : (bt + 1) * 128],
                        )
                        nc.tensor.matmul(
                            out=hT[:, (bt * 2 + 1) * 128 : (bt * 2 + 2) * 128],
                            lhsT=w1dB[:],
                            rhs=sT[:, bt * 128 : (bt + 1) * 128],
                        )
                    nc.scalar.activation(
                        out=hs[:, fs],
                        in_=hT[:, fs],
                        func=mybir.ActivationFunctionType.Relu,
                        bias=b1_2[:],
                    )

                    # w2 + scores matmuls and the q copy (+b2 on the ACT bias
                    # port) per half: consecutive tiles stay decoupled in PSUM
                    for hb in hbs:
                        q = q_psum.tile([2, 512], F32, tag="q")
                        nc.tensor.matmul(
                            out=q[:],
                            lhsT=w2d[:],
                            rhs=hs[:, hb * 512 : (hb + 1) * 512],
                            start=True,
                            stop=False,
                        )
                        nc.tensor.matmul(
                            out=q[:],
                            lhsT=eye2[:],
                            rhs=scT[:, hb * C : (hb + 1) * C, :].rearrange(
                                "t q p -> t (q p)"
                            ),
                            start=False,
                            stop=True,
                        )
                        nc.scalar.activation(
                            out=qsb[:, hb, :],
                            in_=q[:],
                            func=mybir.ActivationFunctionType.Identity,
                            bias=b2_2[:],
                        )
                q_tiles[it] = qsb

            def out_dma(it, split=False):
                # SP HWDGE, emitted AFTER stage1(it+1)'s scT so the blocking
                # qsb-ready wait never delays the x prefetches by more than
                # one iteration. split=True (last tile): one DMA per half so
                # the tail overlaps the second half's MLP.
                base = it * T * 128
                qsb = q_tiles.pop(it)
                if split:
                    HR = T * 64  # rows per half-tile
                    for hb in range(H):
                        nc.sync.dma_start(
                            out=out[
                                base + hb * HR : base + (hb + 1) * HR, :
                            ].rearrange(
                                "(q two p) one -> two q (p one)", two=2, p=128
                            ),
                            in_=qsb[:, hb, :].rearrange("t (q p) -> t q p", p=128),
                        )
                else:
                    nc.sync.dma_start(
                        out=out[base : base + T * 128, :].rearrange(
                            "(h q two p) one -> two (h q) (p one)", two=2, p=128, h=H
                        ),
                        in_=qsb[:].rearrange("t h (q p) -> t (h q) p", p=128),
                    )

            def staged_tiles():
                prefetch()  # no-op when the niter==1 path already issued it
                exp_tile(0)
                stage1(0)
                if niter == 1:
                    constants_b()
                for i in range(1, NTILES):
                    dma_tile(i + 3)
                    if i + 2 < NTILES:
                        scT_dma(i + 2)
                    exp_tile(i)
                    # stage2 of the previous tile is emitted BEFORE stage1 of
                    # this one to keep each in-order engine queue aligned with
                    # the pipeline order.
                    stage2(i - 1)
                    stage1(i)
                    out_dma(i - 1)
                stage2(NTILES - 1, split=True)
                out_dma(NTILES - 1, split=True)

            if niter == 1:
                staged_tiles()
            else:
                # hardware loop: body emitted once, executed niter times
                # (bench-only path for on-device timing via slope).
                # constants_b outside the loop: the w1 fold is not idempotent.
                constants_b()
                with tc.For_i(0, niter, 1):
                    staged_tiles()

    _split_waits(nc)
    return nc


_CACHE = {}


def host_constants(w1, b1, w2, b2):
    """Precompute the device constant tensors in numpy."""
    import ml_dtypes

    bf16 = ml_dtypes.bfloat16
    w1_a = np.asarray(w1, dtype=np.float32).reshape(IN_DIM, HID)
    b1_a = np.asarray(b1, dtype=np.float32).reshape(HID)
    w2_a = np.asarray(w2, dtype=np.float32).reshape(HID)
    b2_a = np.asarray(b2, dtype=np.float32).reshape(1)

    # Fold the (linear) mean stat slot into W1:
    # w1'[r] = w1[r] + w1[mean_row(r)]/TOPK; mean slots of stat stay zero.
    mean_rows = (np.arange(IN_DIM) // (TOPK + 1)) * (TOPK + 1) + TOPK
    w1eff = w1_a + w1_a[mean_rows] / TOPK

    # block-sparse W1: group gg of the 4-group transpose batch sits at stat
    # rows gg*32..gg*32+20; groups (0,1)->A cols (0:64, 64:128),
    # groups (2,3)->B cols (0:64, 64:128)
    w1dA = np.zeros((128, 128), dtype=np.float32)
    w1dB = np.zeros((128, 128), dtype=np.float32)
    w1dA[0:IN_DIM, 0:HID] = w1eff
    w1dA[32 : 32 + IN_DIM, HID:128] = w1eff
    w1dB[64 : 64 + IN_DIM, 0:HID] = w1eff
    w1dB[96 : 96 + IN_DIM, HID:128] = w1eff

    b1d = np.concatenate([b1_a, b1_a]).reshape(128, 1).astype(np.float32)
    w2d = np.zeros((128, 2), dtype=np.float32)
    w2d[0:HID, 0] = w2_a
    w2d[HID:128, 1] = w2_a
    b2d = np.array([[b2_a[0]], [b2_a[0]]], dtype=np.float32)

    return {
        "w1dA": np.ascontiguousarray(w1dA.astype(bf16)),
        "w1dB": np.ascontiguousarray(w1dB.astype(bf16)),
        "b1d": b1d,
        "w2d": np.ascontiguousarray(w2d.astype(bf16)),
        "b2d": b2d,
        "identd": np.eye(128, dtype=np.float32),
        "eye2d": np.eye(2, dtype=np.float32),
    }


def kernel(scores, pred_corners, w1, b1, w2, b2):
    if "nc" not in _CACHE:
        _CACHE["nc"] = build_kernel()
    nc = _CACHE["nc"]

    pc_full = np.ascontiguousarray(
        pred_corners.reshape(B * L, D).astype(np.float32, copy=False)
    )
    sc_full = np.ascontiguousarray(
        scores.reshape(B * L, 1).astype(np.float32, copy=False)
    )
    consts = host_constants(w1, b1, w2, b2)

    in_maps = []
    for c in range(N_CORES):
        in_maps.append(
            {
                "pc": pc_full[c * R : (c + 1) * R],
                "sc": sc_full[c * R : (c + 1) * R],
                **consts,
            }
        )
    res = run_bass_kernel_spmd(nc, in_maps, core_ids=list(range(N_CORES)))
    outs = [res.results[c]["out"] for c in range(N_CORES)]
    full = np.concatenate(outs, axis=0).reshape(B, L, 1)
    return full
